# revision 14
# baseline (speedup 1.0000x reference)
import os
import sys

for _p in ("/opt/trn_rl_repo",):
    if os.path.isdir(_p) and _p not in sys.path:
        sys.path.insert(0, _p)

import numpy as np
import ml_dtypes

import concourse.bass as bass
import concourse.tile as tile
from concourse import bacc
from concourse import mybir
from concourse import bass_utils
from concourse.alu_op_type import AluOpType

BF16 = ml_dtypes.bfloat16
AF = mybir.ActivationFunctionType

S = 1560
DIM = 1536
NH = 12
HD = 128
CACHE = 4680
NCORES = 8
RPC = S // NCORES
EPS = 1e-6
LOCAL_ATTN_SIZE = 3
SINK_SIZE = 1
MAX_ATTN = 32760 if LOCAL_ATTN_SIZE == -1 else LOCAL_ATTN_SIZE * S

NKC = (CACHE + 127) // 128
TAIL = CACHE - (NKC - 1) * 128
QB = 390

RCHUNKS = [(0, 128), (128, 195)]

_CACHED = {}
LAST_RUNS = []


def _build_launch1():
    nc = bacc.Bacc("TRN2", target_bir_lowering=False, debug=False,
                   num_devices=NCORES, num_swdge_queues=4)
    f32, bf = mybir.dt.float32, mybir.dt.bfloat16

    xt_d = nc.dram_tensor("xt", [128, 12, RPC], bf, kind="ExternalInput")
    w3_d = nc.dram_tensor("w3", [9, 128, 12, 512], bf, kind="ExternalInput")
    cq_d = nc.dram_tensor("cq", [RPC, DIM], bf, kind="ExternalInput")
    sq_d = nc.dram_tensor("sq", [RPC, DIM], bf, kind="ExternalInput")
    ck_d = nc.dram_tensor("ck", [RPC, DIM], bf, kind="ExternalInput")
    sk_d = nc.dram_tensor("sk", [RPC, DIM], bf, kind="ExternalInput")
    out_d = nc.dram_tensor("qkv", [RPC, 3 * DIM], bf, kind="ExternalOutput")

    with tile.TileContext(nc) as tc:
        with (
            tc.tile_pool(name="consts", bufs=1) as consts,
            tc.tile_pool(name="wstream", bufs=3) as wstream,
            tc.tile_pool(name="stage", bufs=1) as stagep,
            tc.tile_pool(name="ps", bufs=4, space="PSUM") as psp,
            tc.tile_pool(name="small", bufs=2) as small,
            tc.tile_pool(name="outs", bufs=1) as outsp,
            tc.tile_pool(name="tmp", bufs=1) as tmpp,
        ):
            xt = consts.tile([128, 12, RPC], bf)
            nc.sync.dma_start(xt[:], xt_d.ap())

            stage = {}
            for ti in range(2):
                for ri, (r0, r1) in enumerate(RCHUNKS):
                    stage[(ti, ri)] = stagep.tile([r1 - r0, DIM], f32,
                                                  tag=f"st{ti}{ri}", name=f"st{ti}{ri}")

            ssq = {}
            for ti in range(2):
                for ri, (r0, r1) in enumerate(RCHUNKS):
                    for ns in range(3):
                        ssq[(ti, ri, ns)] = small.tile(
                            [r1 - r0, 1], f32, tag=f"ssq{ti}{ri}{ns}", name=f"ssq{ti}{ri}{ns}")

            outt = {ri: outsp.tile([r1 - r0, 3 * DIM], bf, tag=f"out{ri}", name=f"out{ri}")
                    for ri, (r0, r1) in enumerate(RCHUNKS)}

            sq_scratch = {ri: tmpp.tile([r1 - r0, 512], bf, tag=f"sqs{ri}", name=f"sqs{ri}")
                          for ri, (r0, r1) in enumerate(RCHUNKS)}

            epsb = consts.tile([128, 1], f32, name="epsb")
            nc.vector.memset(epsb[:], EPS)

            tabs = {}
            tab_specs = [(name, dram, ri)
                         for name, dram in (("cq", cq_d), ("sq", sq_d),
                                            ("ck", ck_d), ("sk", sk_d))
                         for ri in range(len(RCHUNKS))]
            for n in range(9):
                wt = wstream.tile([128, 12, 512], bf, tag="w", name="wt")
                eng = (nc.sync, nc.scalar, nc.gpsimd)[n % 3]
                eng.dma_start(wt[:], w3_d.ap()[n])
                if n >= 1 and tab_specs:
                    name, dram, ri = tab_specs.pop(0)
                    r0, r1 = RCHUNKS[ri]
                    t = consts.tile([r1 - r0, DIM], bf,
                                    tag=f"tab{name}{ri}",
                                    name=f"tab{name}{ri}")
                    (nc.scalar if n % 2 else nc.gpsimd).dma_start(
                        t[:], dram.ap()[r0:r1, :])
                    tabs[(name, ri)] = t
                ti, ns = divmod(n, 3)
                for ri, (r0, r1) in enumerate(RCHUNKS):
                    rs = r1 - r0
                    pr = psp.tile([128, 512], f32, tag="pr", name="pr")
                    for kc in range(12):
                        nc.tensor.matmul(
                            pr[:rs, :],
                            xt[:, kc, r0:r1],
                            wt[:, kc, :],
                            start=(kc == 0),
                            stop=(kc == 11),
                        )
                    if ti < 2:
                        nc.scalar.activation(
                            out=sq_scratch[ri][:rs, :],
                            in_=pr[:rs, :],
                            func=AF.Square,
                            accum_out=ssq[(ti, ri, ns)][:rs, :],
                        )
                        nc.vector.tensor_copy(
                            stage[(ti, ri)][:rs, ns * 512:(ns + 1) * 512],
                            pr[:rs, :],
                        )
                    else:
                        nc.vector.tensor_copy(
                            outt[ri][:rs, 2 * DIM + ns * 512:
                                     2 * DIM + (ns + 1) * 512],
                            pr[:rs, :],
                        )

            for ti, (cn, sn) in ((0, ("cq", "sq")), (1, ("ck", "sk"))):
                for ri, (r0, r1) in enumerate(RCHUNKS):
                    rs = r1 - r0
                    st = stage[(ti, ri)]
                    tot = small.tile([rs, 1], f32, tag=f"tot{ti}{ri}", name=f"tot{ti}{ri}")
                    nc.vector.tensor_tensor(
                        tot[:], ssq[(ti, ri, 0)][:rs, :],
                        ssq[(ti, ri, 1)][:rs, :], AluOpType.add)
                    nc.vector.tensor_tensor(
                        tot[:], tot[:], ssq[(ti, ri, 2)][:rs, :],
                        AluOpType.add)
                    nc.scalar.activation(out=tot[:], in_=tot[:], func=AF.Sqrt,
                                         bias=epsb[:rs, :], scale=1.0 / DIM)
                    nc.vector.reciprocal(out=tot[:], in_=tot[:])
                    if ti == 0:
                        nc.vector.tensor_scalar_mul(
                            tot[:], tot[:], 1.0 / float(np.sqrt(HD)))

                    sw = tmpp.tile([rs, DIM], f32, tag=f"sw{ri}", name=f"sw{ri}")
                    st3 = st[:rs, :].rearrange("p (c two) -> p c two", two=2)
                    sw3 = sw[:rs, :].rearrange("p (c two) -> p c two", two=2)
                    nc.scalar.copy(sw3[:, :, 0], st3[:, :, 1])
                    nc.scalar.copy(sw3[:, :, 1], st3[:, :, 0])
                    t1 = tmpp.tile([rs, DIM], f32, tag=f"t1{ri}", name=f"t1{ri}")
                    nc.vector.tensor_tensor(
                        t1[:], st[:rs, :], tabs[(cn, ri)][:], AluOpType.mult)
                    nc.vector.tensor_tensor(
                        sw[:rs, :], sw[:rs, :], tabs[(sn, ri)][:],
                        AluOpType.mult)
                    nc.vector.tensor_tensor(
                        t1[:], t1[:], sw[:rs, :], AluOpType.add)
                    nc.scalar.activation(
                        out=outt[ri][:rs, ti * DIM:(ti + 1) * DIM],
                        in_=t1[:], func=AF.Copy, scale=tot[:])

            for ri, (r0, r1) in enumerate(RCHUNKS):
                for s, eng in ((0, nc.sync), (1, nc.scalar), (2, nc.gpsimd)):
                    eng.dma_start(
                        out_d.ap()[r0:r1, s * DIM:(s + 1) * DIM],
                        outt[ri][:, s * DIM:(s + 1) * DIM])

    nc.finalize()
    return nc


PACKS = [(0, 4), (4, 3), (7, 4), (11, 3), (14, 4), (18, 3),
         (21, 4), (25, 3), (28, 4), (32, 3), (35, 2)]
UNITS = [(0, 0), (0, 1), (0, 2), (0, 3), (1, 0), (1, 1)]


def _build_launch2():
    nc = bacc.Bacc("TRN2", target_bir_lowering=False, debug=False,
                   num_devices=NCORES, num_swdge_queues=4)
    f32, bf, f16 = mybir.dt.float32, mybir.dt.bfloat16, mybir.dt.float16

    qt_d = nc.dram_tensor("qt", [2, 128, S], bf, kind="ExternalInput")
    kt_d = nc.dram_tensor("kt", [2, 128, NKC * 128], bf, kind="ExternalInput")
    vt_d = nc.dram_tensor("vt", [2, 128, NKC, 128], bf, kind="ExternalInput")
    wo_d = nc.dram_tensor("wo", [2, 128, DIM], bf, kind="ExternalInput")
    out_d = nc.dram_tensor("outp", [2, 780, DIM], f16, kind="ExternalOutput")

    with tile.TileContext(nc) as tc:
        with (
            tc.tile_pool(name="consts", bufs=1) as consts,
            tc.tile_pool(name="ps", bufs=1, space="PSUM") as psp,
            tc.tile_pool(name="pt", bufs=2) as ptp,
            tc.tile_pool(name="padd", bufs=2) as paddp,
            tc.tile_pool(name="small", bufs=2) as smallp,
            tc.tile_pool(name="outs", bufs=4) as outsp,
        ):
            qt0 = consts.tile([128, S], bf, name="qt0")
            qt1 = consts.tile([128, S], bf, name="qt1")
            wo0 = consts.tile([128, DIM], bf, name="wo0")
            wo1 = consts.tile([128, DIM], bf, name="wo1")
            kts = [consts.tile([128, NKC * 128], bf, name=f"kt{lh}")
                   for lh in range(2)]
            vts = [consts.tile([128, NKC, 128], bf, name=f"vt{lh}")
                   for lh in range(2)]
            nc.sync.dma_start(kts[0][:, :512], kt_d.ap()[0][:, :512])
            nc.sync.dma_start(qt0[:, :780], qt_d.ap()[0][:, :780])
            nc.sync.dma_start(kts[0][:, 512:2368], kt_d.ap()[0][:, 512:2368])
            nc.sync.dma_start(kts[0][:, 2368:], kt_d.ap()[0][:, 2368:])
            nc.scalar.dma_start(qt0[:, 780:], qt_d.ap()[0][:, 780:])
            nc.scalar.dma_start(qt1[:, :780], qt_d.ap()[1][:, :780])
            nc.gpsimd.dma_start(vts[0][:, :12, :], vt_d.ap()[0][:, :12, :])
            nc.gpsimd.dma_start(vts[0][:, 12:, :], vt_d.ap()[0][:, 12:, :])
            nc.gpsimd.dma_start(kts[1][:, :2368], kt_d.ap()[1][:, :2368])
            nc.gpsimd.dma_start(kts[1][:, 2368:], kt_d.ap()[1][:, 2368:])
            nc.scalar.dma_start(vts[1][:, :12, :], vt_d.ap()[1][:, :12, :])
            nc.scalar.dma_start(vts[1][:, 12:, :], vt_d.ap()[1][:, 12:, :])
            nc.scalar.dma_start(qt1[:, 780:], qt_d.ap()[1][:, 780:])
            nc.sync.dma_start(wo0[:], wo_d.ap()[0])
            nc.sync.dma_start(wo1[:], wo_d.ap()[1])
            qts = [qt0, qt1]
            wos = [wo0, wo1]

            ones128 = consts.tile([128, 128], f32)
            nc.vector.memset(ones128[:], 1.0)
            sacc_a = consts.tile([128, 6, QB], f32)
            sacc_b = consts.tile([128, 6, QB], f32)
            o3u = consts.tile([128, 6, QB], f32)
            o3 = consts.tile([128, 6, QB], bf)

            wsrc = consts.tile([128, 512], bf, name="wsrc")
            nc.vector.memset(wsrc[:], 0.0)
            wdst = consts.tile([128, 8], bf, name="wdst")
            for wu in range(6):
                wp = psp.tile([128, 512], f32, tag="opsum", name="lpw")
                nc.tensor.matmul(wp[:, 0:512], wsrc[:, :128], wsrc[:],
                                 start=True, stop=True)
                if wu == 0:
                    nc.scalar.activation(out=wdst[:], in_=wp[:, 0:8],
                                         func=AF.Exp)

            def chain(u):
                nc.vector.tensor_tensor(sacc_a[:, u, :], sacc_a[:, u, :],
                                        sacc_b[:, u, :], AluOpType.add)
                dbc = psp.tile([128, QB], f32, tag="lpB", name="dbc")
                nc.tensor.matmul(dbc[:], ones128[:], sacc_a[:, u, :],
                                 start=True, stop=True)
                rbc = smallp.tile([128, QB], f32, tag="rbc", name="rbc")
                nc.vector.reciprocal_approx_fast(out=rbc[:], in_=dbc[:])
                nc.vector.tensor_tensor(o3[:, u, :], o3u[:, u, :], rbc[:],
                                        AluOpType.mult)

            pending = []
            for u, (lh, qb) in enumerate(UNITS):
                qsl = qts[lh][:, qb * QB:(qb + 1) * QB]
                kt_t, vt_t = kts[lh], vts[lh]
                opsum = psp.tile([128, 512], f32, tag="opsum", name="opsum")
                for pi, (j0, m) in enumerate(PACKS):
                    tag = "lpA" if pi % 2 == 0 else "lpB"
                    width = 2048 if pi % 2 == 0 else 1536
                    lp = psp.tile([128, width], f32, tag=tag, name="lp")
                    pt = ptp.tile([128, width], bf, tag=tag + "p", name="pt")
                    for t in range(m):
                        j = j0 + t
                        nc.tensor.matmul(
                            lp[:, t * 512:t * 512 + QB],
                            kt_t[:, j * 128:(j + 1) * 128],
                            qsl,
                            start=True, stop=True)
                    if pi == 2 and pending:
                        pending.pop()()
                    lpv = lp.rearrange("p (b c) -> p b c", c=512)[:, 0:m, 0:QB]
                    ptv = pt.rearrange("p (b c) -> p b c", c=512)[:, 0:m, 0:QB]
                    nc.scalar.activation(out=ptv, in_=lpv, func=AF.Exp)
                    for t in range(m):
                        j = j0 + t
                        nc.tensor.matmul(
                            opsum[:, 0:QB],
                            vt_t[:, j, :],
                            pt[:, t * 512:t * 512 + QB],
                            start=(j == 0), stop=(j == NKC - 1))
                    sa = sacc_a[:, u, :]
                    sb = sacc_b[:, u, :]
                    if m == 4:
                        p1 = paddp.tile([128, QB], bf, tag="padd", name="p1")
                        p2 = paddp.tile([128, QB], bf, tag="padd", name="p2")
                        nc.vector.tensor_tensor(
                            p1[:], pt[:, 0:QB], pt[:, 512:512 + QB],
                            AluOpType.add)
                        nc.vector.tensor_tensor(
                            p2[:], pt[:, 1024:1024 + QB],
                            pt[:, 1536:1536 + QB], AluOpType.add)
                        if pi == 0:
                            nc.vector.tensor_copy(sa, p1[:])
                            nc.gpsimd.tensor_copy(sb, p2[:])
                        else:
                            nc.vector.tensor_tensor(sa, sa, p1[:],
                                                    AluOpType.add)
                            nc.gpsimd.tensor_tensor(sb, sb, p2[:],
                                                    AluOpType.add)
                    elif m == 3:
                        p1 = paddp.tile([128, QB], bf, tag="padd", name="p1")
                        nc.vector.tensor_tensor(
                            p1[:], pt[:, 0:QB], pt[:, 512:512 + QB],
                            AluOpType.add)
                        nc.vector.tensor_tensor(sa, sa, p1[:], AluOpType.add)
                        nc.vector.tensor_tensor(sa, sa, pt[:, 1024:1024 + QB],
                                                AluOpType.add)
                    else:
                        nc.vector.tensor_tensor(sa, sa, pt[:, 0:QB],
                                                AluOpType.add)
                        nc.vector.tensor_tensor(
                            sa[0:TAIL, :], sa[0:TAIL, :],
                            pt[0:TAIL, 512:512 + QB], AluOpType.add)
                nc.vector.tensor_copy(o3u[:, u, :], opsum[:, 0:QB])
                pending.append(lambda uu=u: chain(uu))
            pending.pop()()

            QCH = [(0, 128), (128, 128), (256, 128), (384, 128),
                   (512, 128), (640, 128), (768, 12)]
            o3f = o3.rearrange("p u q -> p (u q)")
            pidx = 0
            for blk in range(2):
                for (q0, qn) in QCH:
                    tag = "lpA" if (pidx % 2 == 0) else "lpB"
                    po = psp.tile([128, 1536], f32, tag=tag, name="po")
                    for cc in range(3):
                        wsl = slice(cc * 512, (cc + 1) * 512)
                        if blk == 0:
                            nc.tensor.matmul(
                                po[:qn, wsl], o3f[:, q0:q0 + qn],
                                wos[0][:, wsl], start=True, stop=False)
                            nc.tensor.matmul(
                                po[:qn, wsl],
                                o3f[:, 4 * QB + q0:4 * QB + q0 + qn],
                                wos[1][:, wsl], start=False, stop=True)
                        else:
                            nc.tensor.matmul(
                                po[:qn, wsl], o3f[:, 780 + q0:780 + q0 + qn],
                                wos[0][:, wsl], start=True, stop=True)
                    outf = outsp.tile([128, 1536], f16, tag="outf",
                                      name="outf")
                    if pidx % 2 == 0:
                        nc.scalar.copy(outf[:qn, :], po[:qn, :])
                    else:
                        nc.vector.tensor_copy(outf[:qn, :], po[:qn, :])
                    nc.sync.dma_start(out_d.ap()[blk][q0:q0 + qn, :],
                                      outf[:qn, :])
                    pidx += 1

    nc.finalize()
    return nc


def _cache_plan(current_start, global_end_index, local_end_index, s, kv_size,
                frame_seqlen):
    current_end = current_start + s
    sink_tokens = SINK_SIZE * frame_seqlen

    kind = np.zeros(kv_size, dtype=np.int64)
    idx = np.arange(kv_size, dtype=np.int64)

    if (LOCAL_ATTN_SIZE != -1 and current_end > global_end_index
            and s + local_end_index > kv_size):
        num_evicted = s + local_end_index - kv_size
        num_rolled = local_end_index - num_evicted - sink_tokens
        src0 = sink_tokens + num_evicted
        kind[sink_tokens:sink_tokens + num_rolled] = \
            kind[src0:src0 + num_rolled]
        idx[sink_tokens:sink_tokens + num_rolled] = \
            idx[src0:src0 + num_rolled]
        new_local_end = (local_end_index + current_end - global_end_index
                         - num_evicted)
    else:
        new_local_end = local_end_index + current_end - global_end_index
    local_start = new_local_end - s
    is_recompute = (current_end <= global_end_index) and (current_start > 0)
    write_start = max(local_start, sink_tokens) if is_recompute \
        else local_start
    off = max(0, write_start - local_start)
    wl = max(0, new_local_end - write_start)
    if wl > 0:
        kind[write_start:new_local_end] = 1
        idx[write_start:new_local_end] = off + np.arange(wl)

    if sink_tokens > 0:
        budget = MAX_ATTN - sink_tokens
        if budget > 0:
            lo = max(sink_tokens, new_local_end - budget)
            sel = np.concatenate([np.arange(sink_tokens),
                                  np.arange(lo, new_local_end)])
        else:
            sel = np.arange(sink_tokens)
    else:
        ws = max(0, new_local_end - MAX_ATTN)
        sel = np.arange(ws, new_local_end)

    k_kind, k_idx = kind[sel], idx[sel]
    old_rows = k_idx[k_kind == 0]
    new_rows = k_idx[k_kind == 1]
    return old_rows, new_rows


def _rope_tables(freqs_real, freqs_imag, f, h, w, start_frame, gq, gk):
    c = HD // 2
    c0 = c - 2 * (c // 3)
    c1 = c // 3
    fr = np.asarray(freqs_real, np.float32)
    fi = np.asarray(freqs_imag, np.float32)
    s = f * h * w
    assert s == S
    fidx = np.arange(s) // (h * w)
    hidx = (np.arange(s) // w) % h
    widx = np.arange(s) % w
    fr_pos = np.concatenate([
        fr[start_frame + fidx][:, :c0],
        fr[hidx][:, c0:c0 + c1],
        fr[widx][:, c0 + c1:c0 + 2 * c1],
    ], axis=1)
    fi_pos = np.concatenate([
        fi[start_frame + fidx][:, :c0],
        fi[hidx][:, c0:c0 + c1],
        fi[widx][:, c0 + c1:c0 + 2 * c1],
    ], axis=1)
    C1 = np.repeat(fr_pos, 2, axis=1)
    Sg = np.empty((s, HD), np.float32)
    Sg[:, 0::2] = -fi_pos
    Sg[:, 1::2] = fi_pos
    C = np.tile(C1, (1, NH))
    Sx = np.tile(Sg, (1, NH))
    gq = np.asarray(gq, np.float32)
    gk = np.asarray(gk, np.float32)
    gq_sw = gq.reshape(-1, 2)[:, ::-1].reshape(-1)
    gk_sw = gk.reshape(-1, 2)[:, ::-1].reshape(-1)
    return (C * gq[None, :], Sx * gq_sw[None, :],
            C * gk[None, :], Sx * gk_sw[None, :])


def kernel(x, cache_k, cache_v, freqs_real, freqs_imag,
           wq, bq, wk, bk, wv, bv, wo, bo, gq, gk,
           f_frames, height, width, current_start, global_end_index,
           local_end_index):
    global LAST_RUNS
    LAST_RUNS = []

    x = np.asarray(x, np.float32)
    cache_k = np.asarray(cache_k, np.float32)
    cache_v = np.asarray(cache_v, np.float32)
    wq = np.asarray(wq, np.float32)
    wk = np.asarray(wk, np.float32)
    wv = np.asarray(wv, np.float32)
    wo = np.asarray(wo, np.float32)
    bo = np.asarray(bo, np.float32)
    f = int(f_frames)
    h = int(height)
    w = int(width)
    current_start = int(current_start)
    global_end_index = int(global_end_index)
    local_end_index = int(local_end_index)

    assert x.shape == (1, S, DIM)
    for b in (bq, bk, bv):
        assert not np.any(np.asarray(b)), "nonzero qkv bias unsupported"

    frame_seqlen = h * w
    start_frame = current_start // frame_seqlen

    Cq, Sq, Ck, Sk = _rope_tables(freqs_real, freqs_imag, f, h, w,
                                  start_frame, gq, gk)
    W_all = np.concatenate([wq, wk, wv], axis=1)
    w3 = np.ascontiguousarray(
        W_all.reshape(12, 128, 9, 512).transpose(2, 1, 0, 3)).astype(BF16)
    xT = x[0].T.astype(BF16)

    nc1 = _CACHED.get("l1")
    if nc1 is None:
        nc1 = _CACHED["l1"] = _build_launch1()

    in_maps1 = []
    for c in range(NCORES):
        r0, r1 = c * RPC, (c + 1) * RPC
        xt_c = np.ascontiguousarray(
            xT[:, r0:r1].reshape(12, 128, RPC).transpose(1, 0, 2))
        in_maps1.append({
            "xt": xt_c,
            "w3": w3,
            "cq": np.ascontiguousarray(Cq[r0:r1]).astype(BF16),
            "sq": np.ascontiguousarray(Sq[r0:r1]).astype(BF16),
            "ck": np.ascontiguousarray(Ck[r0:r1]).astype(BF16),
            "sk": np.ascontiguousarray(Sk[r0:r1]).astype(BF16),
        })
    res1 = bass_utils.run_bass_kernel_spmd(nc1, in_maps1,
                                           core_ids=list(range(NCORES)))
    LAST_RUNS.append(res1)
    qkv = np.concatenate([res1.results[c]["qkv"] for c in range(NCORES)],
                         axis=0)
    Q = qkv[:, :DIM]
    Knew = qkv[:, DIM:2 * DIM]
    Vnew = qkv[:, 2 * DIM:]

    old_rows, new_rows = _cache_plan(current_start, global_end_index,
                                     local_end_index, S, cache_k.shape[1],
                                     frame_seqlen)
    n_keys = len(old_rows) + len(new_rows)
    assert n_keys == CACHE, f"unexpected key count {n_keys}"

    K_eff = np.concatenate([
        cache_k[0, old_rows].reshape(len(old_rows), DIM).astype(BF16),
        Knew[new_rows],
    ], axis=0)
    V_eff = np.concatenate([
        cache_v[0, old_rows].reshape(len(old_rows), DIM).astype(BF16),
        Vnew[new_rows],
    ], axis=0)

    K_pad = np.zeros((NKC * 128, DIM), BF16)
    K_pad[:CACHE] = K_eff
    kth = np.ascontiguousarray(K_pad.T.reshape(NH, HD, NKC * 128))
    V_pad = np.zeros((NKC * 128, DIM), BF16)
    V_pad[:CACHE] = V_eff
    vth = np.ascontiguousarray(
        V_pad.reshape(NKC, 128, NH, HD).transpose(2, 1, 0, 3))
    QT = np.ascontiguousarray(Q.T.reshape(NH, HD, S))
    woh = np.ascontiguousarray(wo.reshape(NH, HD, DIM)).astype(BF16)

    nc2 = _CACHED.get("l2")
    if nc2 is None:
        nc2 = _CACHED["l2"] = _build_launch2()

    in_maps2 = []
    for c in range(NCORES):
        g_lo = (3 * c) // 2
        g_hi = g_lo + 1
        if c % 2 == 0:
            g_full, g_half = g_lo, g_hi
        else:
            g_full, g_half = g_hi, g_lo
        qt_c = np.empty((2, HD, S), BF16)
        for i, g in enumerate((g_full, g_half)):
            if c % 2 == 0:
                qt_c[i] = QT[g]
            else:
                qt_c[i, :, 0:780] = QT[g][:, 780:1560]
                qt_c[i, :, 780:1560] = QT[g][:, 0:780]
        in_maps2.append({
            "qt": qt_c,
            "kt": np.ascontiguousarray(kth[[g_full, g_half]]),
            "vt": np.ascontiguousarray(vth[[g_full, g_half]]),
            "wo": np.ascontiguousarray(woh[[g_full, g_half]]),
        })
    res2 = bass_utils.run_bass_kernel_spmd(nc2, in_maps2,
                                           core_ids=list(range(NCORES)))
    LAST_RUNS.append(res2)

    out = np.zeros((S, DIM), np.float32)
    for c in range(NCORES):
        blk = res2.results[c]["outp"].astype(np.float32)
        if c % 2 == 0:
            out[0:780] += blk[0]
            out[780:1560] += blk[1]
        else:
            out[780:1560] += blk[0]
            out[0:780] += blk[1]
    out += bo.reshape(1, DIM)
    return out.reshape(1, S, DIM)



# revision 15
# speedup vs baseline: 1.1381x; 1.1381x over previous
import os
import sys

for _p in ("/opt/trn_rl_repo",):
    if os.path.isdir(_p) and _p not in sys.path:
        sys.path.insert(0, _p)

import numpy as np
import ml_dtypes

import concourse.bass as bass
import concourse.tile as tile
from concourse import bacc
from concourse import mybir
from concourse import bass_utils
from concourse.alu_op_type import AluOpType

BF16 = ml_dtypes.bfloat16
AF = mybir.ActivationFunctionType

S = 1560
DIM = 1536
NH = 12
HD = 128
CACHE = 4680
NCORES = 8
RPC = S // NCORES
EPS = 1e-6
LOCAL_ATTN_SIZE = 3
SINK_SIZE = 1
MAX_ATTN = 32760 if LOCAL_ATTN_SIZE == -1 else LOCAL_ATTN_SIZE * S

NKC = (CACHE + 127) // 128
TAIL = CACHE - (NKC - 1) * 128
QB = 390

RCHUNKS = [(0, 128), (128, 195)]

_CACHED = {}
LAST_RUNS = []


def _build_launch1():
    nc = bacc.Bacc("TRN2", target_bir_lowering=False, debug=False,
                   num_devices=NCORES, num_swdge_queues=4)
    f32, bf = mybir.dt.float32, mybir.dt.bfloat16

    xt_d = nc.dram_tensor("xt", [128, 12, RPC], bf, kind="ExternalInput")
    w3_d = nc.dram_tensor("w3", [9, 128, 12, 512], bf, kind="ExternalInput")
    cq_d = nc.dram_tensor("cq", [RPC, DIM], bf, kind="ExternalInput")
    sq_d = nc.dram_tensor("sq", [RPC, DIM], bf, kind="ExternalInput")
    ck_d = nc.dram_tensor("ck", [RPC, DIM], bf, kind="ExternalInput")
    sk_d = nc.dram_tensor("sk", [RPC, DIM], bf, kind="ExternalInput")
    out_d = nc.dram_tensor("qkv", [RPC, 3 * DIM], bf, kind="ExternalOutput")

    with tile.TileContext(nc) as tc:
        with (
            tc.tile_pool(name="consts", bufs=1) as consts,
            tc.tile_pool(name="wstream", bufs=3) as wstream,
            tc.tile_pool(name="stage", bufs=1) as stagep,
            tc.tile_pool(name="ps", bufs=4, space="PSUM") as psp,
            tc.tile_pool(name="small", bufs=2) as small,
            tc.tile_pool(name="outs", bufs=1) as outsp,
            tc.tile_pool(name="tmp", bufs=1) as tmpp,
        ):
            xt = consts.tile([128, 12, RPC], bf)
            nc.sync.dma_start(xt[:], xt_d.ap())

            stage = {}
            for ti in range(2):
                for ri, (r0, r1) in enumerate(RCHUNKS):
                    stage[(ti, ri)] = stagep.tile([r1 - r0, DIM], f32,
                                                  tag=f"st{ti}{ri}", name=f"st{ti}{ri}")

            ssq = {}
            for ti in range(2):
                for ri, (r0, r1) in enumerate(RCHUNKS):
                    for ns in range(3):
                        ssq[(ti, ri, ns)] = small.tile(
                            [r1 - r0, 1], f32, tag=f"ssq{ti}{ri}{ns}", name=f"ssq{ti}{ri}{ns}")

            outt = {ri: outsp.tile([r1 - r0, 3 * DIM], bf, tag=f"out{ri}", name=f"out{ri}")
                    for ri, (r0, r1) in enumerate(RCHUNKS)}

            sq_scratch = {ri: tmpp.tile([r1 - r0, 512], bf, tag=f"sqs{ri}", name=f"sqs{ri}")
                          for ri, (r0, r1) in enumerate(RCHUNKS)}

            epsb = consts.tile([128, 1], f32, name="epsb")
            nc.vector.memset(epsb[:], EPS)

            tabs = {}
            tab_specs = [(name, dram, ri)
                         for name, dram in (("cq", cq_d), ("sq", sq_d),
                                            ("ck", ck_d), ("sk", sk_d))
                         for ri in range(len(RCHUNKS))]
            for n in range(9):
                wt = wstream.tile([128, 12, 512], bf, tag="w", name="wt")
                eng = (nc.sync, nc.scalar, nc.gpsimd)[n % 3]
                eng.dma_start(wt[:], w3_d.ap()[n])
                if n >= 1 and tab_specs:
                    name, dram, ri = tab_specs.pop(0)
                    r0, r1 = RCHUNKS[ri]
                    t = consts.tile([r1 - r0, DIM], bf,
                                    tag=f"tab{name}{ri}",
                                    name=f"tab{name}{ri}")
                    (nc.scalar if n % 2 else nc.gpsimd).dma_start(
                        t[:], dram.ap()[r0:r1, :])
                    tabs[(name, ri)] = t
                ti, ns = divmod(n, 3)
                for ri, (r0, r1) in enumerate(RCHUNKS):
                    rs = r1 - r0
                    pr = psp.tile([128, 512], f32, tag="pr", name="pr")
                    for kc in range(12):
                        nc.tensor.matmul(
                            pr[:rs, :],
                            xt[:, kc, r0:r1],
                            wt[:, kc, :],
                            start=(kc == 0),
                            stop=(kc == 11),
                        )
                    if ti < 2:
                        nc.scalar.activation(
                            out=sq_scratch[ri][:rs, :],
                            in_=pr[:rs, :],
                            func=AF.Square,
                            accum_out=ssq[(ti, ri, ns)][:rs, :],
                        )
                        nc.vector.tensor_copy(
                            stage[(ti, ri)][:rs, ns * 512:(ns + 1) * 512],
                            pr[:rs, :],
                        )
                    else:
                        nc.vector.tensor_copy(
                            outt[ri][:rs, 2 * DIM + ns * 512:
                                     2 * DIM + (ns + 1) * 512],
                            pr[:rs, :],
                        )

            for ti, (cn, sn) in ((0, ("cq", "sq")), (1, ("ck", "sk"))):
                for ri, (r0, r1) in enumerate(RCHUNKS):
                    rs = r1 - r0
                    st = stage[(ti, ri)]
                    tot = small.tile([rs, 1], f32, tag=f"tot{ti}{ri}", name=f"tot{ti}{ri}")
                    nc.vector.tensor_tensor(
                        tot[:], ssq[(ti, ri, 0)][:rs, :],
                        ssq[(ti, ri, 1)][:rs, :], AluOpType.add)
                    nc.vector.tensor_tensor(
                        tot[:], tot[:], ssq[(ti, ri, 2)][:rs, :],
                        AluOpType.add)
                    nc.scalar.activation(out=tot[:], in_=tot[:], func=AF.Sqrt,
                                         bias=epsb[:rs, :], scale=1.0 / DIM)
                    nc.vector.reciprocal(out=tot[:], in_=tot[:])
                    if ti == 0:
                        nc.vector.tensor_scalar_mul(
                            tot[:], tot[:], 1.0 / float(np.sqrt(HD)))

                    sw = tmpp.tile([rs, DIM], f32, tag=f"sw{ri}", name=f"sw{ri}")
                    st3 = st[:rs, :].rearrange("p (c two) -> p c two", two=2)
                    sw3 = sw[:rs, :].rearrange("p (c two) -> p c two", two=2)
                    nc.scalar.copy(sw3[:, :, 0], st3[:, :, 1])
                    nc.scalar.copy(sw3[:, :, 1], st3[:, :, 0])
                    t1 = tmpp.tile([rs, DIM], f32, tag=f"t1{ri}", name=f"t1{ri}")
                    nc.vector.tensor_tensor(
                        t1[:], st[:rs, :], tabs[(cn, ri)][:], AluOpType.mult)
                    nc.vector.tensor_tensor(
                        sw[:rs, :], sw[:rs, :], tabs[(sn, ri)][:],
                        AluOpType.mult)
                    nc.vector.tensor_tensor(
                        t1[:], t1[:], sw[:rs, :], AluOpType.add)
                    nc.scalar.activation(
                        out=outt[ri][:rs, ti * DIM:(ti + 1) * DIM],
                        in_=t1[:], func=AF.Copy, scale=tot[:])

            for ri, (r0, r1) in enumerate(RCHUNKS):
                for s, eng in ((0, nc.sync), (1, nc.scalar), (2, nc.gpsimd)):
                    eng.dma_start(
                        out_d.ap()[r0:r1, s * DIM:(s + 1) * DIM],
                        outt[ri][:, s * DIM:(s + 1) * DIM])

    nc.finalize()
    return nc


PACKS = [(0, 4), (4, 3), (7, 4), (11, 3), (14, 4), (18, 3),
         (21, 4), (25, 3), (28, 4), (32, 3), (35, 2)]
UNITS = [(0, 0), (0, 1), (0, 2), (0, 3), (1, 0), (1, 1)]


def _build_launch2():
    nc = bacc.Bacc("TRN2", target_bir_lowering=False, debug=False,
                   num_devices=NCORES, num_swdge_queues=4)
    f32, bf, f16 = mybir.dt.float32, mybir.dt.bfloat16, mybir.dt.float16

    qt_d = nc.dram_tensor("qt", [2, 128, S], bf, kind="ExternalInput")
    kt_d = nc.dram_tensor("kt", [2, 128, NKC * 128], bf, kind="ExternalInput")
    vt_d = nc.dram_tensor("vt", [2, 128, NKC, 128], bf, kind="ExternalInput")
    wo_d = nc.dram_tensor("wo", [2, 128, DIM], bf, kind="ExternalInput")
    out_d = nc.dram_tensor("outp", [2, 780, DIM], f16, kind="ExternalOutput")

    with tile.TileContext(nc) as tc:
        with (
            tc.tile_pool(name="consts", bufs=1) as consts,
            tc.tile_pool(name="ps", bufs=1, space="PSUM") as psp,
            tc.tile_pool(name="pt", bufs=2) as ptp,
            tc.tile_pool(name="padd", bufs=2) as paddp,
            tc.tile_pool(name="small", bufs=2) as smallp,
            tc.tile_pool(name="outs", bufs=4) as outsp,
        ):
            qt0 = consts.tile([128, S], bf, name="qt0")
            qt1 = consts.tile([128, S], bf, name="qt1")
            wo0 = consts.tile([128, DIM], bf, name="wo0")
            wo1 = consts.tile([128, DIM], bf, name="wo1")
            kts = [consts.tile([128, NKC * 128], bf, name=f"kt{lh}")
                   for lh in range(2)]
            vts = [consts.tile([128, NKC, 128], bf, name=f"vt{lh}")
                   for lh in range(2)]
            nc.sync.dma_start(kts[0][:, :512], kt_d.ap()[0][:, :512])
            nc.scalar.dma_start(kts[0][:, 512:1536], kt_d.ap()[0][:, 512:1536])
            nc.sync.dma_start(qt0[:, :780], qt_d.ap()[0][:, :780])
            nc.sync.dma_start(kts[0][:, 1536:3136], kt_d.ap()[0][:, 1536:3136])
            nc.scalar.dma_start(kts[0][:, 3136:], kt_d.ap()[0][:, 3136:])
            nc.scalar.dma_start(qt0[:, 780:], qt_d.ap()[0][:, 780:])
            nc.scalar.dma_start(qt1[:, :780], qt_d.ap()[1][:, :780])
            nc.gpsimd.dma_start(vts[0][:, :12, :], vt_d.ap()[0][:, :12, :])
            nc.gpsimd.dma_start(vts[0][:, 12:, :], vt_d.ap()[0][:, 12:, :])
            nc.gpsimd.dma_start(kts[1][:, :2368], kt_d.ap()[1][:, :2368])
            nc.gpsimd.dma_start(kts[1][:, 2368:], kt_d.ap()[1][:, 2368:])
            nc.scalar.dma_start(vts[1][:, :12, :], vt_d.ap()[1][:, :12, :])
            nc.scalar.dma_start(vts[1][:, 12:, :], vt_d.ap()[1][:, 12:, :])
            nc.scalar.dma_start(qt1[:, 780:], qt_d.ap()[1][:, 780:])
            nc.sync.dma_start(wo0[:], wo_d.ap()[0])
            nc.sync.dma_start(wo1[:], wo_d.ap()[1])
            qts = [qt0, qt1]
            wos = [wo0, wo1]

            ones128 = consts.tile([128, 128], f32)
            nc.vector.memset(ones128[:], 1.0)
            sacc_a = consts.tile([128, 6, QB], f32)
            sacc_b = consts.tile([128, 6, QB], f32)
            o3u = consts.tile([128, 6, QB], f32)
            o3 = consts.tile([128, 6, QB], bf)

            wsrc = consts.tile([128, 512], bf, name="wsrc")
            nc.vector.memset(wsrc[:], 0.0)
            wdst = consts.tile([128, 8], bf, name="wdst")
            for wu in range(6):
                wp = psp.tile([128, 512], f32, tag="opsum", name="lpw")
                nc.tensor.matmul(wp[:, 0:512], wsrc[:, :128], wsrc[:],
                                 start=True, stop=True)
                if wu == 0:
                    nc.scalar.activation(out=wdst[:], in_=wp[:, 0:8],
                                         func=AF.Exp)

            def chain(u):
                dbc = psp.tile([128, QB], f32, tag="lpB", name="dbc")
                nc.tensor.matmul(dbc[:], ones128[:], sacc_a[:, u, :],
                                 start=True, stop=False)
                nc.tensor.matmul(dbc[:], ones128[:], sacc_b[:, u, :],
                                 start=False, stop=True)
                rbc = smallp.tile([128, QB], f32, tag="rbc", name="rbc")
                nc.vector.reciprocal_approx_fast(out=rbc[:], in_=dbc[:])
                nc.vector.tensor_tensor(o3[:, u, :], o3u[:, u, :], rbc[:],
                                        AluOpType.mult)

            pending = []
            for u, (lh, qb) in enumerate(UNITS):
                qsl = qts[lh][:, qb * QB:(qb + 1) * QB]
                kt_t, vt_t = kts[lh], vts[lh]
                opsum = psp.tile([128, 512], f32, tag="opsum", name="opsum")
                for pi, (j0, m) in enumerate(PACKS):
                    tag = "lpA" if pi % 2 == 0 else "lpB"
                    width = 2048 if pi % 2 == 0 else 1536
                    lp = psp.tile([128, width], f32, tag=tag, name="lp")
                    pt = ptp.tile([128, width], bf, tag=tag + "p", name="pt")
                    for t in range(m):
                        j = j0 + t
                        nc.tensor.matmul(
                            lp[:, t * 512:t * 512 + QB],
                            kt_t[:, j * 128:(j + 1) * 128],
                            qsl,
                            start=True, stop=True)
                    lpv = lp.rearrange("p (b c) -> p b c", c=512)[:, 0:m, 0:QB]
                    ptv = pt.rearrange("p (b c) -> p b c", c=512)[:, 0:m, 0:QB]
                    nc.scalar.activation(out=ptv, in_=lpv, func=AF.Exp)
                    for t in range(m):
                        j = j0 + t
                        nc.tensor.matmul(
                            opsum[:, 0:QB],
                            vt_t[:, j, :],
                            pt[:, t * 512:t * 512 + QB],
                            start=(j == 0), stop=(j == NKC - 1))
                    if pi == 2 and pending:
                        pending.pop()()
                    sa = sacc_a[:, u, :]
                    sb = sacc_b[:, u, :]
                    if m == 4:
                        p1 = paddp.tile([128, QB], bf, tag="padd", name="p1")
                        p2 = paddp.tile([128, QB], bf, tag="padd", name="p2")
                        nc.vector.tensor_tensor(
                            p1[:], pt[:, 0:QB], pt[:, 512:512 + QB],
                            AluOpType.add)
                        nc.vector.tensor_tensor(
                            p2[:], pt[:, 1024:1024 + QB],
                            pt[:, 1536:1536 + QB], AluOpType.add)
                        if pi == 0:
                            nc.vector.tensor_copy(sa, p1[:])
                            nc.gpsimd.tensor_copy(sb, p2[:])
                        else:
                            nc.vector.tensor_tensor(sa, sa, p1[:],
                                                    AluOpType.add)
                            nc.gpsimd.tensor_tensor(sb, sb, p2[:],
                                                    AluOpType.add)
                    elif m == 3:
                        p1 = paddp.tile([128, QB], bf, tag="padd", name="p1")
                        nc.vector.tensor_tensor(
                            p1[:], pt[:, 0:QB], pt[:, 512:512 + QB],
                            AluOpType.add)
                        nc.vector.tensor_tensor(sa, sa, p1[:], AluOpType.add)
                        nc.vector.tensor_tensor(sa, sa, pt[:, 1024:1024 + QB],
                                                AluOpType.add)
                    else:
                        nc.vector.tensor_tensor(sa, sa, pt[:, 0:QB],
                                                AluOpType.add)
                        nc.vector.tensor_tensor(
                            sa[0:TAIL, :], sa[0:TAIL, :],
                            pt[0:TAIL, 512:512 + QB], AluOpType.add)
                nc.scalar.copy(o3u[:, u, :], opsum[:, 0:QB])
                pending.append(lambda uu=u: chain(uu))
            pending.pop()()

            QCH = [(0, 128), (128, 128), (256, 128), (384, 128),
                   (512, 128), (640, 128), (768, 12)]
            o3f = o3.rearrange("p u q -> p (u q)")
            pidx = 0
            for blk in range(2):
                for (q0, qn) in QCH:
                    tag = "lpA" if (pidx % 2 == 0) else "lpB"
                    po = psp.tile([128, 1536], f32, tag=tag, name="po")
                    for cc in range(3):
                        wsl = slice(cc * 512, (cc + 1) * 512)
                        if blk == 0:
                            nc.tensor.matmul(
                                po[:qn, wsl], o3f[:, q0:q0 + qn],
                                wos[0][:, wsl], start=True, stop=False)
                            nc.tensor.matmul(
                                po[:qn, wsl],
                                o3f[:, 4 * QB + q0:4 * QB + q0 + qn],
                                wos[1][:, wsl], start=False, stop=True)
                        else:
                            nc.tensor.matmul(
                                po[:qn, wsl], o3f[:, 780 + q0:780 + q0 + qn],
                                wos[0][:, wsl], start=True, stop=True)
                    outf = outsp.tile([128, 1536], f16, tag="outf",
                                      name="outf")
                    if pidx % 2 == 0:
                        nc.scalar.copy(outf[:qn, :], po[:qn, :])
                    else:
                        nc.vector.tensor_copy(outf[:qn, :], po[:qn, :])
                    nc.sync.dma_start(out_d.ap()[blk][q0:q0 + qn, :],
                                      outf[:qn, :])
                    pidx += 1

    nc.finalize()
    return nc


def _cache_plan(current_start, global_end_index, local_end_index, s, kv_size,
                frame_seqlen):
    current_end = current_start + s
    sink_tokens = SINK_SIZE * frame_seqlen

    kind = np.zeros(kv_size, dtype=np.int64)
    idx = np.arange(kv_size, dtype=np.int64)

    if (LOCAL_ATTN_SIZE != -1 and current_end > global_end_index
            and s + local_end_index > kv_size):
        num_evicted = s + local_end_index - kv_size
        num_rolled = local_end_index - num_evicted - sink_tokens
        src0 = sink_tokens + num_evicted
        kind[sink_tokens:sink_tokens + num_rolled] = \
            kind[src0:src0 + num_rolled]
        idx[sink_tokens:sink_tokens + num_rolled] = \
            idx[src0:src0 + num_rolled]
        new_local_end = (local_end_index + current_end - global_end_index
                         - num_evicted)
    else:
        new_local_end = local_end_index + current_end - global_end_index
    local_start = new_local_end - s
    is_recompute = (current_end <= global_end_index) and (current_start > 0)
    write_start = max(local_start, sink_tokens) if is_recompute \
        else local_start
    off = max(0, write_start - local_start)
    wl = max(0, new_local_end - write_start)
    if wl > 0:
        kind[write_start:new_local_end] = 1
        idx[write_start:new_local_end] = off + np.arange(wl)

    if sink_tokens > 0:
        budget = MAX_ATTN - sink_tokens
        if budget > 0:
            lo = max(sink_tokens, new_local_end - budget)
            sel = np.concatenate([np.arange(sink_tokens),
                                  np.arange(lo, new_local_end)])
        else:
            sel = np.arange(sink_tokens)
    else:
        ws = max(0, new_local_end - MAX_ATTN)
        sel = np.arange(ws, new_local_end)

    k_kind, k_idx = kind[sel], idx[sel]
    old_rows = k_idx[k_kind == 0]
    new_rows = k_idx[k_kind == 1]
    return old_rows, new_rows


def _rope_tables(freqs_real, freqs_imag, f, h, w, start_frame, gq, gk):
    c = HD // 2
    c0 = c - 2 * (c // 3)
    c1 = c // 3
    fr = np.asarray(freqs_real, np.float32)
    fi = np.asarray(freqs_imag, np.float32)
    s = f * h * w
    assert s == S
    fidx = np.arange(s) // (h * w)
    hidx = (np.arange(s) // w) % h
    widx = np.arange(s) % w
    fr_pos = np.concatenate([
        fr[start_frame + fidx][:, :c0],
        fr[hidx][:, c0:c0 + c1],
        fr[widx][:, c0 + c1:c0 + 2 * c1],
    ], axis=1)
    fi_pos = np.concatenate([
        fi[start_frame + fidx][:, :c0],
        fi[hidx][:, c0:c0 + c1],
        fi[widx][:, c0 + c1:c0 + 2 * c1],
    ], axis=1)
    C1 = np.repeat(fr_pos, 2, axis=1)
    Sg = np.empty((s, HD), np.float32)
    Sg[:, 0::2] = -fi_pos
    Sg[:, 1::2] = fi_pos
    C = np.tile(C1, (1, NH))
    Sx = np.tile(Sg, (1, NH))
    gq = np.asarray(gq, np.float32)
    gk = np.asarray(gk, np.float32)
    gq_sw = gq.reshape(-1, 2)[:, ::-1].reshape(-1)
    gk_sw = gk.reshape(-1, 2)[:, ::-1].reshape(-1)
    return (C * gq[None, :], Sx * gq_sw[None, :],
            C * gk[None, :], Sx * gk_sw[None, :])


def kernel(x, cache_k, cache_v, freqs_real, freqs_imag,
           wq, bq, wk, bk, wv, bv, wo, bo, gq, gk,
           f_frames, height, width, current_start, global_end_index,
           local_end_index):
    global LAST_RUNS
    LAST_RUNS = []

    x = np.asarray(x, np.float32)
    cache_k = np.asarray(cache_k, np.float32)
    cache_v = np.asarray(cache_v, np.float32)
    wq = np.asarray(wq, np.float32)
    wk = np.asarray(wk, np.float32)
    wv = np.asarray(wv, np.float32)
    wo = np.asarray(wo, np.float32)
    bo = np.asarray(bo, np.float32)
    f = int(f_frames)
    h = int(height)
    w = int(width)
    current_start = int(current_start)
    global_end_index = int(global_end_index)
    local_end_index = int(local_end_index)

    assert x.shape == (1, S, DIM)
    for b in (bq, bk, bv):
        assert not np.any(np.asarray(b)), "nonzero qkv bias unsupported"

    frame_seqlen = h * w
    start_frame = current_start // frame_seqlen

    Cq, Sq, Ck, Sk = _rope_tables(freqs_real, freqs_imag, f, h, w,
                                  start_frame, gq, gk)
    W_all = np.concatenate([wq, wk, wv], axis=1)
    w3 = np.ascontiguousarray(
        W_all.reshape(12, 128, 9, 512).transpose(2, 1, 0, 3)).astype(BF16)
    xT = x[0].T.astype(BF16)

    nc1 = _CACHED.get("l1")
    if nc1 is None:
        nc1 = _CACHED["l1"] = _build_launch1()

    in_maps1 = []
    for c in range(NCORES):
        r0, r1 = c * RPC, (c + 1) * RPC
        xt_c = np.ascontiguousarray(
            xT[:, r0:r1].reshape(12, 128, RPC).transpose(1, 0, 2))
        in_maps1.append({
            "xt": xt_c,
            "w3": w3,
            "cq": np.ascontiguousarray(Cq[r0:r1]).astype(BF16),
            "sq": np.ascontiguousarray(Sq[r0:r1]).astype(BF16),
            "ck": np.ascontiguousarray(Ck[r0:r1]).astype(BF16),
            "sk": np.ascontiguousarray(Sk[r0:r1]).astype(BF16),
        })
    res1 = bass_utils.run_bass_kernel_spmd(nc1, in_maps1,
                                           core_ids=list(range(NCORES)))
    LAST_RUNS.append(res1)
    qkv = np.concatenate([res1.results[c]["qkv"] for c in range(NCORES)],
                         axis=0)
    Q = qkv[:, :DIM]
    Knew = qkv[:, DIM:2 * DIM]
    Vnew = qkv[:, 2 * DIM:]

    old_rows, new_rows = _cache_plan(current_start, global_end_index,
                                     local_end_index, S, cache_k.shape[1],
                                     frame_seqlen)
    n_keys = len(old_rows) + len(new_rows)
    assert n_keys == CACHE, f"unexpected key count {n_keys}"

    K_eff = np.concatenate([
        cache_k[0, old_rows].reshape(len(old_rows), DIM).astype(BF16),
        Knew[new_rows],
    ], axis=0)
    V_eff = np.concatenate([
        cache_v[0, old_rows].reshape(len(old_rows), DIM).astype(BF16),
        Vnew[new_rows],
    ], axis=0)

    K_pad = np.zeros((NKC * 128, DIM), BF16)
    K_pad[:CACHE] = K_eff
    kth = np.ascontiguousarray(K_pad.T.reshape(NH, HD, NKC * 128))
    V_pad = np.zeros((NKC * 128, DIM), BF16)
    V_pad[:CACHE] = V_eff
    vth = np.ascontiguousarray(
        V_pad.reshape(NKC, 128, NH, HD).transpose(2, 1, 0, 3))
    QT = np.ascontiguousarray(Q.T.reshape(NH, HD, S))
    woh = np.ascontiguousarray(wo.reshape(NH, HD, DIM)).astype(BF16)

    nc2 = _CACHED.get("l2")
    if nc2 is None:
        nc2 = _CACHED["l2"] = _build_launch2()

    in_maps2 = []
    for c in range(NCORES):
        g_lo = (3 * c) // 2
        g_hi = g_lo + 1
        if c % 2 == 0:
            g_full, g_half = g_lo, g_hi
        else:
            g_full, g_half = g_hi, g_lo
        qt_c = np.empty((2, HD, S), BF16)
        for i, g in enumerate((g_full, g_half)):
            if c % 2 == 0:
                qt_c[i] = QT[g]
            else:
                qt_c[i, :, 0:780] = QT[g][:, 780:1560]
                qt_c[i, :, 780:1560] = QT[g][:, 0:780]
        in_maps2.append({
            "qt": qt_c,
            "kt": np.ascontiguousarray(kth[[g_full, g_half]]),
            "vt": np.ascontiguousarray(vth[[g_full, g_half]]),
            "wo": np.ascontiguousarray(woh[[g_full, g_half]]),
        })
    res2 = bass_utils.run_bass_kernel_spmd(nc2, in_maps2,
                                           core_ids=list(range(NCORES)))
    LAST_RUNS.append(res2)

    out = np.zeros((S, DIM), np.float32)
    for c in range(NCORES):
        blk = res2.results[c]["outp"].astype(np.float32)
        if c % 2 == 0:
            out[0:780] += blk[0]
            out[780:1560] += blk[1]
        else:
            out[780:1560] += blk[0]
            out[0:780] += blk[1]
    out += bo.reshape(1, DIM)
    return out.reshape(1, S, DIM)

